# revision 11
# baseline (speedup 1.0000x reference)
"""DTW L1 loss kernel for Trainium2 (8 NeuronCores, batch-data-parallel).

loss = sum_{b,p,c} |preds[b, path_i[b,p], c] - targets[b, path_j[b,p], c]| / (B*S)

Hybrid two-pipeline design per NeuronCore (32 batches):

  Pipeline G (GPSIMD, batches 0..NG-1): stock ap_gather at its measured
  33.4 cyc/idx floor — tables pre-packed as uint32 bf16-pairs, x16 partition
  replication, index tiles preloaded, ring-buffered gather outputs, DVE
  subtract + abs-reduce. (Identical to the tuned single-pipeline kernel.)

  Pipeline R (PE/DVE/Act, batches NG..31): radix gather. idx = 128*q + r.
  The whole per-batch table lives in ONE 128x128 bf16 PE weight tile
  W[r, 2q+c] = table[128q + r, c]. Host ships fp8 one-hot R[r_p, p] and
  channel-pair mask M[2q_p+c, p]. Per 512-column chunk:
    E = W^T @ R_chunk            (PE, all 64 q-candidates for each pair)
    mE = M_chunk * E             (DVE, zeroes every row but the right q)
    D += ones_red^T @ mE_pred    (PE M=2 reduce; accumulate -targ likewise)
  so D[c, p] = pred[i_p, c] - targ[j_p, c] lands positionally, then
  Act computes |D| with a fused free-axis accumulation. The ragged pad
  column has all-zero masks on both sides, contributing exactly 0.

  Pipelines share no engines (G: GPSIMD+DVE-lite, R: PE+DVE+Act+DMA), so
  they overlap; the final partition-reduce matmul merges both partials.
"""

import os
import numpy as np

# Problem geometry (hardcoded per contract).
B, S, C = 256, 8192, 2
P = 16383
N_CORES = 8
B_NC = B // N_CORES            # 32 batches per NeuronCore
P_PAD = P + 1                  # 16384: pad each batch's path with one idx 0
N_OFF = int(os.environ.get("DTW_NOFF", "24"))   # batches on pipeline R (mult of 8)
NG = B_NC - N_OFF              # batches on pipeline G
CHUNK = 512                    # pipeline R matmul chunk columns
GROUP = 2048                   # pipeline R DMA group columns

_NC_CACHE = {}
LAST_RESULTS = None


def _build_nc(b_nc=B_NC, s=S, p_pad=P_PAD, ci=4096):
    """Build the Bass module for one NeuronCore's shard (SPMD across 8)."""
    from contextlib import ExitStack

    import concourse.bacc as bacc
    import concourse.tile as tile
    from concourse import mybir

    f32 = mybir.dt.float32
    bf16 = mybir.dt.bfloat16
    fp8 = mybir.dt.float8e4
    u32 = mybir.dt.uint32
    i16 = mybir.dt.int16

    ng = b_nc - N_OFF
    assert ng % 8 == 0 and N_OFF % 8 == 0
    rounds = ng // 8
    assert p_pad % 16 == 0 and ci % 16 == 0 and p_pad % ci == 0
    pcols = p_pad // 16            # idx cols per round per path
    cicols = ci // 16              # idx cols per gather call
    chunks = p_pad // ci
    assert s * 4 // 4 <= 2**15     # ap_gather per-partition table limit (words)
    ngroups = p_pad // GROUP
    cpg = GROUP // CHUNK           # matmul chunks per DMA group

    nc = bacc.Bacc("TRN2")

    # --- pipeline G inputs (uint32-packed bf16 pairs, x16 replicated) ---
    preds_t = nc.dram_tensor("preds", [ng * 16, s], u32, kind="ExternalInput")
    targets_t = nc.dram_tensor("targets", [ng * 16, s], u32, kind="ExternalInput")
    pi_t = nc.dram_tensor("pi", [rounds * 128, pcols], i16, kind="ExternalInput")
    pj_t = nc.dram_tensor("pj", [rounds * 128, pcols], i16, kind="ExternalInput")
    # --- pipeline R inputs ---
    wp_t = nc.dram_tensor("wp", [N_OFF * 128, 128], bf16, kind="ExternalInput")
    wt_t = nc.dram_tensor("wt", [N_OFF * 128, 128], bf16, kind="ExternalInput")
    rp_t = nc.dram_tensor("rp", [N_OFF * 128, p_pad], fp8, kind="ExternalInput")
    rt_t = nc.dram_tensor("rt", [N_OFF * 128, p_pad], fp8, kind="ExternalInput")
    mp_t = nc.dram_tensor("mp", [N_OFF * 128, p_pad], fp8, kind="ExternalInput")
    mt_t = nc.dram_tensor("mt", [N_OFF * 128, p_pad], fp8, kind="ExternalInput")
    ored_t = nc.dram_tensor("ored", [128, 2], bf16, kind="ExternalInput")
    oredn_t = nc.dram_tensor("oredn", [128, 2], bf16, kind="ExternalInput")
    out_t = nc.dram_tensor("out", [1, 1], f32, kind="ExternalOutput")

    def load_table_replicated(tab_tile, dram, r):
        # all 16 partitions of Q7 core b hold batch (r*8+b)'s table
        nc.sync.dma_start(out=tab_tile[:], in_=dram[r * 128:(r + 1) * 128, :])

    with tile.TileContext(nc) as tc, ExitStack() as ctx:
        singles = ctx.enter_context(tc.tile_pool(name="singles", bufs=1))
        tab_pool = ctx.enter_context(tc.tile_pool(name="tabs", bufs=2))
        gout = ctx.enter_context(tc.tile_pool(name="gout", bufs=3))

        partials = singles.tile([128, rounds * chunks], f32)
        corr = singles.tile([128, rounds], f32)

        # index tiles live for the whole kernel (tiny); round 0's idx_i goes
        # first so the first gather's deps clear fastest
        idx_i = singles.tile([128, rounds * pcols], i16)
        idx_j = singles.tile([128, rounds * pcols], i16)
        nc.sync.dma_start(out=idx_i[:, 0:pcols], in_=pi_t[0:128, :])

        tab_p0 = tab_pool.tile([128, s], u32, tag="tab")
        load_table_replicated(tab_p0, preds_t, 0)
        tab_t0 = tab_pool.tile([128, s], u32, tag="tab")
        load_table_replicated(tab_t0, targets_t, 0)

        nc.sync.dma_start(out=idx_j[:, 0:pcols], in_=pj_t[0:128, :])
        for r in range(1, rounds):
            nc.sync.dma_start(
                out=idx_i[:, r * pcols:(r + 1) * pcols],
                in_=pi_t[r * 128:(r + 1) * 128, :],
            )
            nc.sync.dma_start(
                out=idx_j[:, r * pcols:(r + 1) * pcols],
                in_=pj_t[r * 128:(r + 1) * 128, :],
            )

        # ---- pipeline R setup ----
        ored = singles.tile([128, 2], bf16)
        nc.sync.dma_start(out=ored[:], in_=ored_t[:, :])
        oredn = singles.tile([128, 2], bf16)
        nc.sync.dma_start(out=oredn[:], in_=oredn_t[:, :])
        nchk_off = N_OFF * (p_pad // CHUNK)
        acc_off = singles.tile([2, max(nchk_off, 1)], f32)
        wpool = ctx.enter_context(tc.tile_pool(name="wpool", bufs=2))
        rmpool = ctx.enter_context(tc.tile_pool(name="rmpool", bufs=3))
        mepool = ctx.enter_context(tc.tile_pool(name="mepool", bufs=6))
        scrpool = ctx.enter_context(tc.tile_pool(name="scr", bufs=3))
        epool = ctx.enter_context(tc.tile_pool(name="exp", bufs=6, space="PSUM"))
        dpool = ctx.enter_context(tc.tile_pool(name="dp", bufs=2, space="PSUM"))

        pending = []  # software-pipelined (mex, mey, col) awaiting d/ACT

        def emit_pending():
            mex, mey, col = pending.pop(0)
            d = dpool.tile([2, CHUNK], f32, tag="d")
            nc.tensor.matmul(
                out=d[:], lhsT=ored[:], rhs=mex[:],
                start=True, stop=False,
            )
            nc.tensor.matmul(
                out=d[:], lhsT=oredn[:], rhs=mey[:],
                start=False, stop=True,
            )
            scr = scrpool.tile([2, CHUNK], f32, tag="scr")
            nc.scalar.activation(
                out=scr[:], in_=d[:],
                func=mybir.ActivationFunctionType.Abs,
                accum_out=acc_off[:, col:col + 1],
            )

        def pipeline_r(ob):
            wp = wpool.tile([128, 128], bf16, tag="wp")
            nc.sync.dma_start(out=wp[:], in_=wp_t[ob * 128:(ob + 1) * 128, :])
            wt = wpool.tile([128, 128], bf16, tag="wt")
            nc.sync.dma_start(out=wt[:], in_=wt_t[ob * 128:(ob + 1) * 128, :])
            for g in range(ngroups):
                gsl = slice(g * GROUP, (g + 1) * GROUP)
                rsl = slice(ob * 128, (ob + 1) * 128)
                rpg = rmpool.tile([128, GROUP], fp8, tag="rp")
                nc.sync.dma_start(out=rpg[:], in_=rp_t[rsl, gsl])
                mpg = rmpool.tile([128, GROUP], fp8, tag="mp")
                nc.sync.dma_start(out=mpg[:], in_=mp_t[rsl, gsl])
                rtg = rmpool.tile([128, GROUP], fp8, tag="rt")
                nc.sync.dma_start(out=rtg[:], in_=rt_t[rsl, gsl])
                mtg = rmpool.tile([128, GROUP], fp8, tag="mt")
                nc.sync.dma_start(out=mtg[:], in_=mt_t[rsl, gsl])
                for k in range(cpg):
                    ksl = slice(k * CHUNK, (k + 1) * CHUNK)
                    ex = epool.tile([128, CHUNK], f32, tag="e")
                    nc.tensor.matmul(
                        out=ex[:], lhsT=wp[:], rhs=rpg[:, ksl],
                        start=True, stop=True,
                    )
                    ey = epool.tile([128, CHUNK], f32, tag="e")
                    nc.tensor.matmul(
                        out=ey[:], lhsT=wt[:], rhs=rtg[:, ksl],
                        start=True, stop=True,
                    )
                    # lag-2: emit an older chunk's d/ACT now that its DVE
                    # products are ready and this chunk's E-matmuls queued
                    while len(pending) > 1:
                        emit_pending()
                    mex = mepool.tile([128, CHUNK], bf16, tag="me")
                    nc.vector.tensor_tensor(
                        out=mex[:], in0=ex[:], in1=mpg[:, ksl],
                        op=mybir.AluOpType.mult,
                    )
                    mey = mepool.tile([128, CHUNK], bf16, tag="me")
                    nc.vector.tensor_tensor(
                        out=mey[:], in0=ey[:], in1=mtg[:, ksl],
                        op=mybir.AluOpType.mult,
                    )
                    col = ob * (p_pad // CHUNK) + g * cpg + k
                    pending.append((mex, mey, col))

        # ---- interleave: issue R batches between G rounds so both start ----
        off_iter = list(range(N_OFF))
        per_iter = -(-N_OFF // max(rounds * chunks, 1))

        def issue_off(nb):
            for _ in range(nb):
                if off_iter:
                    pipeline_r(off_iter.pop(0))

        for r in range(rounds):
            if r == 0:
                tab_p, tab_t = tab_p0, tab_t0
            else:
                tab_p = tab_pool.tile([128, s], u32, tag="tab")
                load_table_replicated(tab_p, preds_t, r)
                tab_t = tab_pool.tile([128, s], u32, tag="tab")
                load_table_replicated(tab_t, targets_t, r)

            # pad-pair correction: both padded lists point at row 0
            cd = singles.tile([128, C], f32, tag="cd")
            nc.vector.tensor_tensor(
                out=cd[:],
                in0=tab_p[:, 0:1].bitcast(bf16),
                in1=tab_t[:, 0:1].bitcast(bf16),
                op=mybir.AluOpType.subtract,
            )
            nc.vector.tensor_reduce(
                out=corr[:, r:r + 1], in_=cd[:],
                axis=mybir.AxisListType.X, op=mybir.AluOpType.add,
                apply_absolute_value=True,
            )

            for k in range(chunks):
                csl = slice(r * pcols + k * cicols, r * pcols + (k + 1) * cicols)
                gp = gout.tile([128, ci], u32, tag="g")
                nc.gpsimd.ap_gather(
                    out_ap=gp[:], in_ap=tab_p[:], idxs_ap=idx_i[:, csl],
                    channels=128, num_elems=s, d=1, num_idxs=ci,
                )
                gt = gout.tile([128, ci], u32, tag="g")
                nc.gpsimd.ap_gather(
                    out_ap=gt[:], in_ap=tab_t[:], idxs_ap=idx_j[:, csl],
                    channels=128, num_elems=s, d=1, num_idxs=ci,
                )
                dflat = gp[:].bitcast(bf16)
                nc.vector.tensor_tensor(
                    out=dflat, in0=dflat, in1=gt[:].bitcast(bf16),
                    op=mybir.AluOpType.subtract,
                )
                nc.vector.tensor_reduce(
                    out=partials[:, r * chunks + k:r * chunks + k + 1],
                    in_=dflat, axis=mybir.AxisListType.X,
                    op=mybir.AluOpType.add, apply_absolute_value=True,
                )
                issue_off(per_iter)
        issue_off(N_OFF)
        while pending:
            emit_pending()

        total = singles.tile([128, 1], f32)
        nc.vector.tensor_reduce(
            out=total[:], in_=partials[:],
            axis=mybir.AxisListType.X, op=mybir.AluOpType.add,
        )
        corrtot = singles.tile([128, 1], f32)
        nc.vector.tensor_reduce(
            out=corrtot[:], in_=corr[:],
            axis=mybir.AxisListType.X, op=mybir.AluOpType.add,
        )
        nc.vector.tensor_tensor(
            out=total[:], in0=total[:], in1=corrtot[:],
            op=mybir.AluOpType.subtract,
        )
        # pipeline R: reduce per-chunk |D| accums, x16 to match G's replication
        offtot = singles.tile([2, 1], f32)
        nc.vector.tensor_reduce(
            out=offtot[:], in_=acc_off[:],
            axis=mybir.AxisListType.X, op=mybir.AluOpType.add,
        )
        nc.vector.tensor_scalar(
            offtot[:], offtot[:], 16.0, None, op0=mybir.AluOpType.mult,
        )
        nc.vector.tensor_tensor(
            out=total[0:2, :], in0=total[0:2, :], in1=offtot[:],
            op=mybir.AluOpType.add,
        )
        ones = singles.tile([128, 1], f32)
        nc.vector.memset(ones[:], 1.0)
        acc_psum = dpool.tile([1, 1], f32, tag="d")
        nc.tensor.matmul(
            out=acc_psum[:], lhsT=total[:], rhs=ones[:], start=True, stop=True
        )
        scalar = singles.tile([1, 1], f32)
        nc.vector.tensor_scalar(
            scalar[:], acc_psum[:], 1.0 / (16.0 * B * S), None,
            op0=mybir.AluOpType.mult,
        )
        nc.sync.dma_start(out=out_t[:], in_=scalar[:])

    nc.finalize()
    return nc


def _get_nc():
    key = "full"
    if key not in _NC_CACHE:
        _NC_CACHE[key] = _build_nc()
    return _NC_CACHE[key]


def _pack_bf16_pairs(arr):
    """[b, s, 2] f32 -> [b, s] uint32 of packed round-to-nearest-even bf16."""
    import ml_dtypes

    bf = arr.astype(ml_dtypes.bfloat16).view(np.uint16).astype(np.uint32)
    return (bf[..., 0] | (bf[..., 1] << 16)).copy()


def _wrap_paths(arr, b_nc, p, p_pad):
    """[b_nc, p] int -> padded int16 wrapped rows [rounds*128, p_pad//16]."""
    pad = np.zeros((b_nc, p_pad - p), dtype=arr.dtype)
    a = np.concatenate([arr, pad], axis=1)  # [b_nc, p_pad]
    pcols = p_pad // 16
    a = a.reshape(b_nc, pcols, 16).transpose(0, 2, 1)  # [b_nc, 16, pcols]
    return np.ascontiguousarray(a.reshape(b_nc // 8, 8 * 16, pcols)).reshape(
        (b_nc // 8) * 128, pcols
    ).astype(np.int16)


def _radix_side(tab_f32, idx):
    """One pipeline-R side: weights [128,128] bf16, one-hot R and mask M fp8.

    tab_f32: [S, 2] f32 table; idx: [P] indices. Pad col P_PAD-1 is all-zero.
    """
    import ml_dtypes

    fp8 = ml_dtypes.float8_e4m3
    w = np.ascontiguousarray(
        tab_f32.astype(ml_dtypes.bfloat16).reshape(64, 128, 2).transpose(1, 0, 2)
        .reshape(128, 128)
    )
    full = np.zeros(P_PAD, dtype=np.int64)
    full[:P] = idx
    q, r = full >> 7, full & 127
    cols = np.arange(P_PAD)
    R = np.zeros((128, P_PAD), dtype=fp8)
    R[r[:P], cols[:P]] = 1.0
    M = np.zeros((128, P_PAD), dtype=fp8)
    M[2 * q[:P], cols[:P]] = 1.0
    M[2 * q[:P] + 1, cols[:P]] = 1.0
    return w, R, M


def _shard_inputs(preds, targets, path_i, path_j):
    import ml_dtypes

    preds = np.asarray(preds, dtype=np.float32)
    targets = np.asarray(targets, dtype=np.float32)
    packed_p = _pack_bf16_pairs(preds)
    packed_t = _pack_bf16_pairs(targets)
    path_i = np.asarray(path_i)
    path_j = np.asarray(path_j)

    ored = np.zeros((128, 2), dtype=ml_dtypes.bfloat16)
    ored[0::2, 0] = 1.0
    ored[1::2, 1] = 1.0
    oredn = -ored

    in_maps = []
    for c in range(N_CORES):
        b0 = c * B_NC
        g0, g1 = b0, b0 + NG            # pipeline G batches
        o0, o1 = b0 + NG, b0 + B_NC     # pipeline R batches
        wps, wts, rps, rts, mps, mts = [], [], [], [], [], []
        for b in range(o0, o1):
            w, R, M = _radix_side(preds[b], path_i[b])
            wps.append(w); rps.append(R); mps.append(M)
            w, R, M = _radix_side(targets[b], path_j[b])
            wts.append(w); rts.append(R); mts.append(M)
        in_maps.append(
            {
                "preds": np.repeat(packed_p[g0:g1], 16, axis=0),
                "targets": np.repeat(packed_t[g0:g1], 16, axis=0),
                "pi": _wrap_paths(path_i[g0:g1], NG, P, P_PAD),
                "pj": _wrap_paths(path_j[g0:g1], NG, P, P_PAD),
                "wp": np.concatenate(wps, axis=0),
                "wt": np.concatenate(wts, axis=0),
                "rp": np.concatenate(rps, axis=0),
                "rt": np.concatenate(rts, axis=0),
                "mp": np.concatenate(mps, axis=0),
                "mt": np.concatenate(mts, axis=0),
                "ored": ored,
                "oredn": oredn,
            }
        )
    return in_maps


def kernel(**inputs):
    global LAST_RESULTS
    from concourse.bass_utils import run_bass_kernel_spmd

    in_maps = _shard_inputs(
        inputs["preds"], inputs["targets"], inputs["path_i"], inputs["path_j"]
    )
    nc = _get_nc()
    trace = bool(int(os.environ.get("DTW_TRACE", "0")))
    res = run_bass_kernel_spmd(nc, in_maps, list(range(N_CORES)), trace=trace)
    LAST_RESULTS = res
    total = np.float32(0.0)
    for r in res.results:
        total = total + np.float32(r["out"].reshape(-1)[0])
    return np.float32(total)
